# revision 34
# baseline (speedup 1.0000x reference)
"""VQ EuclideanCodebook kernel for Trainium2 (Bass/Tile), 8-core data-parallel.

Math: embed_ind = argmax_c( -(|x|^2 - 2 x.e_c + |e_c|^2) ) = argmax_c( x.e_c - 0.5|e_c|^2 )
      quantize  = embed[embed_ind]

Device (per core, 4096 tokens = 32 tiles of 128):
  - scores[tok, code] accumulated in fp32 PSUM from bf16 matmuls:
      2 rank-1 bias matmuls (hi/lo split of -0.5*|e|^2, exact to ~2e-3)
      + 4 accumulated K=128 matmuls over the 512-dim contraction.
  - ScalarE evicts PSUM -> SBUF; VectorE max (top-8 values) + max_index
    (argmax), written straight into persistent SBUF accumulators.
  - GpSimd indirect-DMA gathers embed[idx] rows from DRAM into a persistent
    SBUF strip; per-tile stores go to per-tile DRAM tensors.
  (Every DMA is structured to carry at most ONE sync wait: this walrus
   build's DIRECT2D pseudo rejects multi-wait DMAs, so no SBUF slot that a
   DMA writes is ever reused, and no DRAM tensor is DMA-written twice.)
Host: pre-transposes/casts x, embed to bf16 tile layouts; afterwards re-scores
  the few tokens whose device top-2 gap < DELTA exactly in f64 (bf16 rounding
  moves scores by <~0.3; true top-2 gaps below DELTA are the only ambiguous
  ones, and the minimum true gap in this regime is ~3e-4, so the exact f64
  rescore reproduces the f32 reference argmax).
"""

import hashlib
from contextlib import ExitStack

import ml_dtypes
import numpy as np

P = 128          # partitions / tokens per tile
D = 512          # embedding dim
K = 4096         # codebook size
DC = 4           # contraction chunks (4 x 128)
NB = 8           # PSUM banks (512 codes each)
BANK = 512
N_CORES = 8
TOK_PER_CORE = 4096
N_TILES = TOK_PER_CORE // P   # 32
DELTA = 1.0      # host re-score threshold on device top-2 gap

BF16 = ml_dtypes.bfloat16


def _tag_width(n_tiles, repeats):
    """Neuron's compile cache keys on an HLO fingerprint that ignores the
    bass program embedded in backend_config — two different bass programs
    with identical I/O shapes collide and silently run a stale NEFF.  A
    dummy input whose SHAPE hashes kernel source + build params forces a
    distinct fingerprint per program version."""
    with open(__file__, "rb") as f:
        src = f.read()
    h = hashlib.sha256(src + repr((n_tiles, repeats)).encode()).digest()
    return 1 + (int.from_bytes(h[:4], "little") % 59999)


def _probe_tag(probe):
    return 1 + (int.from_bytes(
        hashlib.sha256(repr(tuple(sorted(probe))).encode()).digest()[:2],
        "little") % 59)


def build_program(n_tiles=N_TILES, repeats=1, probe=()):
    """probe: subset of {"no_bias_lo", "no_maxindex", "no_scans"} — A/B
    variants for bottleneck attribution; outputs are wrong under probes."""
    import concourse.bass as bass
    import concourse.tile as tile
    from concourse import bacc, mybir

    dt = mybir.dt
    # Bacc (not raw Bass): its compile() pass legalizes multi-wait
    # instructions, which this walrus build rejects otherwise.
    nc = bacc.Bacc("TRN2", target_bir_lowering=False, debug=False,
                   num_devices=N_CORES)

    xt = nc.dram_tensor("xt", [n_tiles, P, DC * P], dt.bfloat16,
                        kind="ExternalInput").ap()
    et = nc.dram_tensor("et", [P, DC * K], dt.bfloat16,
                        kind="ExternalInput").ap()
    bias = nc.dram_tensor("bias", [2, K], dt.bfloat16,
                          kind="ExternalInput").ap()
    emb = nc.dram_tensor("emb", [K, D], dt.float32,
                         kind="ExternalInput").ap()
    nc.dram_tensor("tag", [_probe_tag(probe), _tag_width(n_tiles, repeats)],
                   dt.uint8, kind="ExternalInput")
    idx_o = nc.dram_tensor("idx8", [P, n_tiles * 8], dt.uint32,
                           kind="ExternalOutput").ap()
    val_o = nc.dram_tensor("val8", [P, n_tiles * 8], dt.float32,
                           kind="ExternalOutput").ap()
    q_o = nc.dram_tensor("quant", [n_tiles * P, D], dt.float32,
                         kind="ExternalOutput").ap()

    with tile.TileContext(nc) as tc, ExitStack() as ctx:
        const = ctx.enter_context(tc.tile_pool(name="const", bufs=1))
        # bufs=n_tiles: fresh SBUF slot per iteration so DMA-written tiles
        # are never reused (keeps every DMA at <=1 sync wait).
        xpool = ctx.enter_context(tc.tile_pool(name="x", bufs=n_tiles))
        spool = ctx.enter_context(tc.tile_pool(name="scores", bufs=2))
        psum = ctx.enter_context(tc.tile_pool(name="psum", bufs=NB,
                                              space="PSUM"))

        et_sb = const.tile([P, DC * K], dt.bfloat16)
        nc.sync.dma_start(et_sb[:], et[:])
        # bias rows (hi, lo) replicated on partition pairs {32j, 32j+1}: the
        # 8 per-bank bias matmuls are K=2 and use only 2 PE rows each, so
        # with tile_position=(32j, 0) four of them run concurrently in
        # distinct 32-row groups of the array.
        bias_sb = const.tile([P, K], dt.bfloat16)
        nc.gpsimd.memset(bias_sb[:], 0.0)
        for j in range(4):
            nc.sync.dma_start(bias_sb[32 * j:32 * j + 2, :], bias[:, :])
        ones_sb = const.tile([P, P], dt.bfloat16)
        nc.gpsimd.memset(ones_sb[:], 1.0)

        idx_acc = const.tile([P, n_tiles * 8], dt.uint32)
        val_acc = const.tile([P, n_tiles * 8], dt.float32)
        q_acc = const.tile([P, n_tiles * D], dt.float32)

        for rep, t in ((r, t) for r in range(repeats)
                       for t in range(n_tiles)):
            x_sb = xpool.tile([P, DC * P], dt.bfloat16, tag="x",
                              name=f"x_sb_{rep}_{t}")
            nc.sync.dma_start(x_sb[:], xt[t, :, :])

            pbs = [psum.tile([P, BANK], dt.float32, tag="ps",
                             name=f"ps_{rep}_{t}_{b}")
                   for b in range(NB)]
            if "no_bias" not in probe:
                for b in range(NB):
                    j = 32 * (b % 4)
                    nc.tensor.matmul(pbs[b][:],
                                     lhsT=ones_sb[j:j + 2, :],
                                     rhs=bias_sb[j:j + 2,
                                                 b * BANK:(b + 1) * BANK],
                                     start=True, stop=False,
                                     tile_position=(j, 0))
            for c in range(DC):
                for b in range(NB):
                    nc.tensor.matmul(
                        pbs[b][:],
                        lhsT=x_sb[:, c * P:(c + 1) * P],
                        rhs=et_sb[:, c * K + b * BANK:c * K + (b + 1) * BANK],
                        start=(c == 0 and "no_bias" in probe),
                        stop=(c == DC - 1))

            s_sb = spool.tile([P, K], dt.float32, tag="s")
            for b in range(NB):
                nc.scalar.copy(s_sb[:, b * BANK:(b + 1) * BANK], pbs[b][:])

            v8 = val_acc[:, t * 8:(t + 1) * 8]
            i8 = idx_acc[:, t * 8:(t + 1) * 8]
            if "no_scans" in probe:
                if rep == 0 and t == 0:
                    nc.gpsimd.memset(idx_acc[:], 0)
                    nc.gpsimd.memset(val_acc[:], 0)
            else:
                nc.vector.max(
                    v8, s_sb[:, :BANK] if "short_max" in probe else s_sb[:])
                if "no_maxindex" in probe:
                    if rep == 0 and t == 0:
                        nc.gpsimd.memset(idx_acc[:], 0)
                elif "short_maxindex" in probe:
                    nc.vector.max_index(i8, v8, s_sb[:, :BANK])
                else:
                    nc.vector.max_index(i8, v8, s_sb[:])

            g = q_acc[:, t * D:(t + 1) * D]
            bc = dict(bounds_check=K - 1, oob_is_err=False) if probe else {}
            nc.gpsimd.indirect_dma_start(
                out=g, out_offset=None, in_=emb[:],
                in_offset=bass.IndirectOffsetOnAxis(
                    ap=idx_acc[:, t * 8:t * 8 + 1], axis=0), **bc)
            nc.sync.dma_start(q_o[t * P:(t + 1) * P, :], g)

        nc.sync.dma_start(idx_o[:, :], idx_acc[:])
        nc.sync.dma_start(val_o[:, :], val_acc[:])

    nc.compile()
    return nc


def prep_inputs(x, embed, n_tiles=N_TILES, repeats=1, probe=()):
    """Host-side layout prep. Returns per-core input maps."""
    tag = np.zeros((_probe_tag(probe), _tag_width(n_tiles, repeats)), np.uint8)
    xf = np.ascontiguousarray(x.reshape(-1, D))
    n_tok = xf.shape[0]
    tok_per_core = n_tok // N_CORES

    e_sq = (embed.astype(np.float64) ** 2).sum(axis=1)
    bias_f32 = (-0.5 * e_sq).astype(np.float32)
    bias_hi = bias_f32.astype(BF16)
    bias_lo = (bias_f32 - bias_hi.astype(np.float32)).astype(BF16)
    bias_arr = np.ascontiguousarray(np.stack([bias_hi, bias_lo]))  # [2, K]

    # et[p, c*K + code] = embed[code, c*128 + p]
    emb4 = embed.reshape(K, DC, P)
    et = np.ascontiguousarray(emb4.transpose(2, 1, 0).reshape(P, DC * K)
                              .astype(BF16))
    emb_f32 = np.ascontiguousarray(embed.astype(np.float32))

    in_maps = []
    for core in range(N_CORES):
        shard = xf[core * tok_per_core:(core + 1) * tok_per_core]
        xs = shard.reshape(tok_per_core // P, P, DC, P)   # [t, j, c, p]
        xtc = np.ascontiguousarray(xs.transpose(0, 3, 2, 1)
                                   .reshape(tok_per_core // P, P, DC * P)
                                   .astype(BF16))
        in_maps.append({"xt": xtc, "et": et, "bias": bias_arr,
                        "emb": emb_f32, "tag": tag})
    return in_maps


def gather_outputs(results, n_tiles=N_TILES):
    """Per-core output dicts -> (idx [N], val8 [N,8], quant [N,512])."""
    idx_l, val_l, q_l = [], [], []
    for r in results:
        acc_i = r["idx8"].reshape(P, n_tiles, 8)
        acc_v = r["val8"].reshape(P, n_tiles, 8)
        idx_l.append(acc_i.transpose(1, 0, 2).reshape(-1, 8)[:, 0])
        val_l.append(acc_v.transpose(1, 0, 2).reshape(-1, 8))
        q_l.append(r["quant"])
    return (np.concatenate(idx_l).astype(np.int64),
            np.concatenate(val_l, axis=0),
            np.concatenate(q_l, axis=0))


def kernel(x, embed):
    from concourse.bass_utils import run_bass_kernel_spmd

    x = np.asarray(x)
    embed = np.asarray(embed)
    orig_shape = x.shape
    xf = x.reshape(-1, D).astype(np.float32)

    in_maps = prep_inputs(x, embed)
    nc = build_program(N_TILES)
    res = run_bass_kernel_spmd(nc, in_maps, list(range(N_CORES)))
    idx, val8, quant = gather_outputs(res.results)

    # Host rescue of near-ties: exact rescore of tokens with small device gap.
    gap = val8[:, 0].astype(np.float64) - val8[:, 1].astype(np.float64)
    flagged = np.where(gap < DELTA)[0]
    if flagged.size:
        e64 = embed.astype(np.float64)
        e_sq = (e64 ** 2).sum(axis=1)
        s = 2.0 * (xf[flagged].astype(np.float64) @ e64.T) - e_sq[None, :]
        true_idx = s.argmax(axis=1)
        idx[flagged] = true_idx
        quant[flagged] = embed[true_idx].astype(np.float32)

    embed_ind = idx.astype(np.int32).reshape(orig_shape[:-1])
    quantize = quant.reshape(orig_shape).astype(np.float32)
    return embed_ind, quantize


# revision 36
# speedup vs baseline: 1.2500x; 1.2500x over previous
"""VQ EuclideanCodebook kernel for Trainium2 (Bass/Tile), 8-core data-parallel.

Math: embed_ind = argmax_c( -(|x|^2 - 2 x.e_c + |e_c|^2) ) = argmax_c( x.e_c - 0.5|e_c|^2 )
      quantize  = embed[embed_ind]

Device (per core, 4096 tokens = 32 tiles of 128):
  - scores[tok, code] accumulated in fp32 PSUM from bf16 matmuls:
      2 rank-1 bias matmuls (hi/lo split of -0.5*|e|^2, exact to ~2e-3)
      + 4 accumulated K=128 matmuls over the 512-dim contraction.
  - ScalarE evicts PSUM -> SBUF; VectorE max (top-8 values) + max_index
    (argmax), written straight into persistent SBUF accumulators.
  - GpSimd indirect-DMA gathers embed[idx] rows from DRAM into a persistent
    SBUF strip; per-tile stores go to per-tile DRAM tensors.
  (Every DMA is structured to carry at most ONE sync wait: this walrus
   build's DIRECT2D pseudo rejects multi-wait DMAs, so no SBUF slot that a
   DMA writes is ever reused, and no DRAM tensor is DMA-written twice.)
Host: pre-transposes/casts x, embed to bf16 tile layouts; afterwards re-scores
  the few tokens whose device top-2 gap < DELTA exactly in f64 (bf16 rounding
  moves scores by <~0.3; true top-2 gaps below DELTA are the only ambiguous
  ones, and the minimum true gap in this regime is ~3e-4, so the exact f64
  rescore reproduces the f32 reference argmax).
"""

import hashlib
from contextlib import ExitStack

import ml_dtypes
import numpy as np

P = 128          # partitions / tokens per tile
D = 512          # embedding dim
K = 4096         # codebook size
DC = 4           # contraction chunks (4 x 128)
NB = 8           # PSUM banks (512 codes each)
BANK = 512
N_CORES = 8
TOK_PER_CORE = 4096
N_TILES = TOK_PER_CORE // P   # 32
DELTA = 1.0      # host re-score threshold on device top-2 gap

BF16 = ml_dtypes.bfloat16


def _tag_width(n_tiles, repeats):
    """Neuron's compile cache keys on an HLO fingerprint that ignores the
    bass program embedded in backend_config — two different bass programs
    with identical I/O shapes collide and silently run a stale NEFF.  A
    dummy input whose SHAPE hashes kernel source + build params forces a
    distinct fingerprint per program version."""
    with open(__file__, "rb") as f:
        src = f.read()
    h = hashlib.sha256(src + repr((n_tiles, repeats)).encode()).digest()
    return 1 + (int.from_bytes(h[:4], "little") % 59999)


def _probe_tag(probe):
    return 1 + (int.from_bytes(
        hashlib.sha256(repr(tuple(sorted(probe))).encode()).digest()[:2],
        "little") % 59)


def build_program(n_tiles=N_TILES, repeats=1, probe=()):
    """probe: subset of {"no_bias_lo", "no_maxindex", "no_scans"} — A/B
    variants for bottleneck attribution; outputs are wrong under probes."""
    import concourse.bass as bass
    import concourse.tile as tile
    from concourse import bacc, mybir

    dt = mybir.dt
    # Bacc (not raw Bass): its compile() pass legalizes multi-wait
    # instructions, which this walrus build rejects otherwise.
    nc = bacc.Bacc("TRN2", target_bir_lowering=False, debug=False,
                   num_devices=N_CORES)

    xt = nc.dram_tensor("xt", [n_tiles, P, DC * P], dt.bfloat16,
                        kind="ExternalInput").ap()
    et = nc.dram_tensor("et", [P, DC * K], dt.bfloat16,
                        kind="ExternalInput").ap()
    bias = nc.dram_tensor("bias", [2, K], dt.bfloat16,
                          kind="ExternalInput").ap()
    emb = nc.dram_tensor("emb", [K, D], dt.float32,
                         kind="ExternalInput").ap()
    nc.dram_tensor("tag", [_probe_tag(probe), _tag_width(n_tiles, repeats)],
                   dt.uint8, kind="ExternalInput")
    idx_o = nc.dram_tensor("idx8", [P, n_tiles * 8], dt.uint32,
                           kind="ExternalOutput").ap()
    val_o = nc.dram_tensor("val8", [P, n_tiles * 8], dt.float32,
                           kind="ExternalOutput").ap()
    q_o = nc.dram_tensor("quant", [n_tiles * P, D], dt.float32,
                         kind="ExternalOutput").ap()

    with tile.TileContext(nc) as tc, ExitStack() as ctx:
        const = ctx.enter_context(tc.tile_pool(name="const", bufs=1))
        # bufs=n_tiles: fresh SBUF slot per iteration so DMA-written tiles
        # are never reused (keeps every DMA at <=1 sync wait).
        xpool = ctx.enter_context(tc.tile_pool(name="x", bufs=n_tiles))
        spool = ctx.enter_context(tc.tile_pool(name="scores", bufs=2))
        psum = ctx.enter_context(tc.tile_pool(name="psum", bufs=NB,
                                              space="PSUM"))

        et_sb = const.tile([P, DC * K], dt.bfloat16)
        nc.sync.dma_start(et_sb[:], et[:])
        # bias rows (hi, lo) live on partitions 0-1 of a full-K tile (rest
        # zeros): the bias matmul is then shaped exactly like a score matmul
        # (K=128), which the PE pipelines at streaming rate — both a skinny
        # K=2 matmul and a 4-way row-tiled variant measured slower.
        bias_sb = const.tile([P, K], dt.bfloat16)
        nc.gpsimd.memset(bias_sb[:], 0.0)
        nc.sync.dma_start(bias_sb[:2, :], bias[:, :])
        ones_sb = const.tile([P, P], dt.bfloat16)
        nc.gpsimd.memset(ones_sb[:], 1.0)

        idx_acc = const.tile([P, n_tiles * 8], dt.uint32)
        val_acc = const.tile([P, n_tiles * 8], dt.float32)
        q_acc = const.tile([P, n_tiles * D], dt.float32)

        for rep, t in ((r, t) for r in range(repeats)
                       for t in range(n_tiles)):
            x_sb = xpool.tile([P, DC * P], dt.bfloat16, tag="x",
                              name=f"x_sb_{rep}_{t}")
            nc.sync.dma_start(x_sb[:], xt[t, :, :])

            pbs = [psum.tile([P, BANK], dt.float32, tag="ps",
                             name=f"ps_{rep}_{t}_{b}")
                   for b in range(NB)]
            if "no_bias" not in probe:
                for b in range(NB):
                    nc.tensor.matmul(pbs[b][:], lhsT=ones_sb[:],
                                     rhs=bias_sb[:, b * BANK:(b + 1) * BANK],
                                     start=True, stop=False)
            for c in range(DC):
                for b in range(NB):
                    nc.tensor.matmul(
                        pbs[b][:],
                        lhsT=x_sb[:, c * P:(c + 1) * P],
                        rhs=et_sb[:, c * K + b * BANK:c * K + (b + 1) * BANK],
                        start=(c == 0 and "no_bias" in probe),
                        stop=(c == DC - 1))

            s_sb = spool.tile([P, K], dt.float32, tag="s")
            for b in range(NB):
                nc.scalar.copy(s_sb[:, b * BANK:(b + 1) * BANK], pbs[b][:])

            v8 = val_acc[:, t * 8:(t + 1) * 8]
            i8 = idx_acc[:, t * 8:(t + 1) * 8]
            if "no_scans" in probe:
                if rep == 0 and t == 0:
                    nc.gpsimd.memset(idx_acc[:], 0)
                    nc.gpsimd.memset(val_acc[:], 0)
            else:
                nc.vector.max(
                    v8, s_sb[:, :BANK] if "short_max" in probe else s_sb[:])
                if "no_maxindex" in probe:
                    if rep == 0 and t == 0:
                        nc.gpsimd.memset(idx_acc[:], 0)
                elif "short_maxindex" in probe:
                    nc.vector.max_index(i8, v8, s_sb[:, :BANK])
                else:
                    nc.vector.max_index(i8, v8, s_sb[:])

            g = q_acc[:, t * D:(t + 1) * D]
            bc = dict(bounds_check=K - 1, oob_is_err=False) if probe else {}
            nc.gpsimd.indirect_dma_start(
                out=g, out_offset=None, in_=emb[:],
                in_offset=bass.IndirectOffsetOnAxis(
                    ap=idx_acc[:, t * 8:t * 8 + 1], axis=0), **bc)
            nc.sync.dma_start(q_o[t * P:(t + 1) * P, :], g)

        nc.sync.dma_start(idx_o[:, :], idx_acc[:])
        nc.sync.dma_start(val_o[:, :], val_acc[:])

    nc.compile()
    return nc


def prep_inputs(x, embed, n_tiles=N_TILES, repeats=1, probe=()):
    """Host-side layout prep. Returns per-core input maps."""
    tag = np.zeros((_probe_tag(probe), _tag_width(n_tiles, repeats)), np.uint8)
    xf = np.ascontiguousarray(x.reshape(-1, D))
    n_tok = xf.shape[0]
    tok_per_core = n_tok // N_CORES

    e_sq = (embed.astype(np.float64) ** 2).sum(axis=1)
    bias_f32 = (-0.5 * e_sq).astype(np.float32)
    bias_hi = bias_f32.astype(BF16)
    bias_lo = (bias_f32 - bias_hi.astype(np.float32)).astype(BF16)
    bias_arr = np.ascontiguousarray(np.stack([bias_hi, bias_lo]))  # [2, K]

    # et[p, c*K + code] = embed[code, c*128 + p]
    emb4 = embed.reshape(K, DC, P)
    et = np.ascontiguousarray(emb4.transpose(2, 1, 0).reshape(P, DC * K)
                              .astype(BF16))
    emb_f32 = np.ascontiguousarray(embed.astype(np.float32))

    in_maps = []
    for core in range(N_CORES):
        shard = xf[core * tok_per_core:(core + 1) * tok_per_core]
        xs = shard.reshape(tok_per_core // P, P, DC, P)   # [t, j, c, p]
        xtc = np.ascontiguousarray(xs.transpose(0, 3, 2, 1)
                                   .reshape(tok_per_core // P, P, DC * P)
                                   .astype(BF16))
        in_maps.append({"xt": xtc, "et": et, "bias": bias_arr,
                        "emb": emb_f32, "tag": tag})
    return in_maps


def gather_outputs(results, n_tiles=N_TILES):
    """Per-core output dicts -> (idx [N], val8 [N,8], quant [N,512])."""
    idx_l, val_l, q_l = [], [], []
    for r in results:
        acc_i = r["idx8"].reshape(P, n_tiles, 8)
        acc_v = r["val8"].reshape(P, n_tiles, 8)
        idx_l.append(acc_i.transpose(1, 0, 2).reshape(-1, 8)[:, 0])
        val_l.append(acc_v.transpose(1, 0, 2).reshape(-1, 8))
        q_l.append(r["quant"])
    return (np.concatenate(idx_l).astype(np.int64),
            np.concatenate(val_l, axis=0),
            np.concatenate(q_l, axis=0))


def kernel(x, embed):
    from concourse.bass_utils import run_bass_kernel_spmd

    x = np.asarray(x)
    embed = np.asarray(embed)
    orig_shape = x.shape
    xf = x.reshape(-1, D).astype(np.float32)

    in_maps = prep_inputs(x, embed)
    nc = build_program(N_TILES)
    res = run_bass_kernel_spmd(nc, in_maps, list(range(N_CORES)))
    idx, val8, quant = gather_outputs(res.results)

    # Host rescue of near-ties: exact rescore of tokens with small device gap.
    gap = val8[:, 0].astype(np.float64) - val8[:, 1].astype(np.float64)
    flagged = np.where(gap < DELTA)[0]
    if flagged.size:
        e64 = embed.astype(np.float64)
        e_sq = (e64 ** 2).sum(axis=1)
        s = 2.0 * (xf[flagged].astype(np.float64) @ e64.T) - e_sq[None, :]
        true_idx = s.argmax(axis=1)
        idx[flagged] = true_idx
        quant[flagged] = embed[true_idx].astype(np.float32)

    embed_ind = idx.astype(np.int32).reshape(orig_shape[:-1])
    quantize = quant.reshape(orig_shape).astype(np.float32)
    return embed_ind, quantize
